# revision 15
# baseline (speedup 1.0000x reference)
"""Causal self-attention (B=2, S=4096, D=512, H=8) on 8 Trainium2 NeuronCores.

Sharding: tensor-parallel over heads. Core h computes head h for both batch
elements: QKV projections for its head, causal flash attention, and its
partial (unnormalized) o_proj contribution y_h = U_h @ Wo[h*64:(h+1)*64, :]
plus the per-query softmax denominators L_h. The host computes
sum_h(y_h / L_h) + bo.

Per-core structure (hd = 64, S = 4096, 32 k-tiles of 128 per batch):
  - xT [512, 8192] (host-pretransposed x) streams in as [128f, 4c, 512t]
    tiles; QK projection matmul (lhsT = [Wq_h | Wk_h] chunk) produces
    psum [Q.T; K.T] per 512-token block; V.T separately, then PE-transposed
    to V natural (bf16).
  - QT2 [128, 4096]/batch: Q.T duplicated in both partition halves (dup via
    SBUF->SBUF DMA). KT2 [128, 2048]/batch: K.T pair-packed -- even k-tiles
    in partitions 0:64, odd k-tiles in partitions 64:128.
  - Scores: per k-tile pair, TWO K=64 matmuls at tile_position (0,0) and
    (64,0) (disjoint PE row groups -> run concurrently): S.T chunk psum
    [128, 2, 512]. One ACT exp call [128, 1024] PSUM->SBUF produces P.T in
    bf16; diagonal chunks get a 0/1 causal mask multiply on DVE.
  - AV (bf16): U'[65, 512] += V'_kt.T @ P.T_kt with V' = [V | ones]; row 64
    accumulates L. AV for chunk j is emitted after the scores of chunk j+1.
  - o_proj: U dup'd to both partition halves; per q-subtile pair, TWO K=64
    matmuls at row groups 0/64 against Wo_h (host-duplicated into both
    halves); y out in bf16, 256 rows per DMA, unnormalized, plus L.
  - proj of block t+1 and o_proj of block t-1 are emitted as small "filler"
    pieces between attention chunks so the Scalar engine (exp, the
    throughput floor at ~1.15us per 1024-col chunk) never starves and the
    PE never idles long enough to re-throttle (HAM).

Matmuls: scores/projections/o_proj in float32r (~1.6e-4), AV in bf16.
"""

import sys

for _p in ("/opt/trn_rl_repo", "/root/.axon_site/_ro/trn_rl_repo"):
    if _p not in sys.path:
        sys.path.insert(0, _p)

from collections import deque

import numpy as np

import concourse.bass as bass
import concourse.mybir as mybir
import concourse.tile as tile
from concourse import bacc
from concourse.bass_utils import run_bass_kernel_spmd

B = 2
S = 4096
D = 512
H = 8
HD = 64
TOK = B * S          # 8192
NKT = S // 128       # 32 k-tiles per batch
SCALE = HD ** -0.5

F32 = mybir.dt.float32
F32R = mybir.dt.float32r
BF16 = mybir.dt.bfloat16

_CACHE = {}


def _build():
    nc = bacc.Bacc("TRN2", target_bir_lowering=False, debug=False, num_devices=8)

    xt_d = nc.dram_tensor("xt", [D, TOK], F32R, kind="ExternalInput")
    wqk_d = nc.dram_tensor("wqk", [D, 128], F32R, kind="ExternalInput")
    wv_d = nc.dram_tensor("wv", [D, HD], F32R, kind="ExternalInput")
    wo_d = nc.dram_tensor("wo", [128, D], F32R, kind="ExternalInput")
    bqk_d = nc.dram_tensor("bqk", [128, 1], F32, kind="ExternalInput")
    bv_d = nc.dram_tensor("bv", [HD, 1], F32, kind="ExternalInput")
    mask_d = nc.dram_tensor("mask", [128, 4, 512], BF16, kind="ExternalInput")
    identb_d = nc.dram_tensor("identb", [64, 64], BF16, kind="ExternalInput")
    onesb_d = nc.dram_tensor("onesb", [128, NKT], BF16, kind="ExternalInput")
    y_d = nc.dram_tensor("y", [TOK, D], BF16, kind="ExternalOutput")
    l_d = nc.dram_tensor("l", [TOK], F32R, kind="ExternalOutput")

    xt_r = xt_d.ap().rearrange("(c p) t -> p c t", p=128)      # [128, 4, 8192]
    wqk_r = wqk_d.ap().rearrange("(c p) m -> p c m", p=128)    # [128, 4, 128]
    wv_r = wv_d.ap().rearrange("(c p) m -> p c m", p=128)      # [128, 4, 64]

    with tile.TileContext(nc) as tc:
        import contextlib

        with contextlib.ExitStack() as ctx:
            singles = ctx.enter_context(tc.tile_pool(name="singles", bufs=1))
            xpool = ctx.enter_context(tc.tile_pool(name="xt", bufs=3))
            ptpool = ctx.enter_context(tc.tile_pool(name="pt", bufs=5))
            upool = ctx.enter_context(tc.tile_pool(name="usb", bufs=2))
            ypool = ctx.enter_context(tc.tile_pool(name="ysb", bufs=4))
            kstpool = ctx.enter_context(tc.tile_pool(name="kst", bufs=2))
            ktspool = ctx.enter_context(tc.tile_pool(name="kts", bufs=2))

            ps_st = ctx.enter_context(
                tc.tile_pool(name="ps_st", bufs=3, space="PSUM")
            )
            ps_u = ctx.enter_context(tc.tile_pool(name="ps_u", bufs=1, space="PSUM"))
            ps_misc = ctx.enter_context(
                tc.tile_pool(name="ps_misc", bufs=1, space="PSUM")
            )

            # --- persistent per-batch activation buffers ---------------
            qt2 = [
                singles.tile([128, S], F32R, tag=f"qt2_{b}", name=f"qt2_{b}")
                for b in range(B)
            ]
            kt2 = [
                singles.tile([128, S // 2], F32R, tag=f"kt2_{b}", name=f"kt2_{b}")
                for b in range(B)
            ]
            vp = [
                singles.tile([128, NKT * 65], BF16, tag=f"vp_{b}", name=f"vp_{b}")
                for b in range(B)
            ]
            l_acc = singles.tile([1, TOK], F32R, name="l_acc")

            blocks = [(b, t) for b in range(B) for t in range(8)]
            xt_tiles = {}

            def start_xt(i):
                if i >= len(blocks):
                    return
                b, tb = blocks[i]
                t0 = b * S + tb * 512
                xt_sb = xpool.tile([128, 4, 512], F32R, tag="xt")
                nc.gpsimd.dma_start(out=xt_sb, in_=xt_r[:, :, t0 : t0 + 512])
                xt_tiles[i] = xt_sb

            # --- constants / weights (xt(0) first: it gates proj(0)) ---
            start_xt(0)
            wqk_sb = singles.tile([128, 4, 128], F32R)
            wv_sb = singles.tile([128, 4, HD], F32R)
            wo_sb = singles.tile([128, D], F32R)
            bqk_sb = singles.tile([128, 1], F32)
            bv_sb = singles.tile([HD, 1], F32)
            mask_sb = singles.tile([128, 4, 512], BF16)
            identb = singles.tile([64, 64], BF16)
            ones_sb = singles.tile([128, NKT], BF16, name="ones_sb")
            nc.sync.dma_start(out=wqk_sb, in_=wqk_r)
            nc.sync.dma_start(out=bqk_sb, in_=bqk_d.ap())
            nc.sync.dma_start(out=wv_sb, in_=wv_r)
            nc.sync.dma_start(out=bv_sb, in_=bv_d.ap())
            nc.sync.dma_start(out=identb, in_=identb_d.ap())
            nc.sync.dma_start(out=ones_sb, in_=onesb_d.ap())
            nc.sync.dma_start(out=mask_sb, in_=mask_d.ap())
            nc.sync.dma_start(out=wo_sb, in_=wo_d.ap())
            start_xt(1)

            def vp_ones(b):
                nc.vector.tensor_copy(
                    vp[b].rearrange("p (t c) -> p t c", c=65)[:, :, 64:65],
                    ones_sb.rearrange("p (t c) -> p t c", c=1),
                )

            def proj_pieces(i):
                """Projections for block i as a list of small emit-closures."""
                b, tb = blocks[i]
                st_ = {}

                def qk(c0):
                    def f():
                        if c0 == 0:
                            st_["qk"] = ps_misc.tile(
                                [128, 512], F32, tag="m", name="qkps"
                            )
                        for c in (c0, c0 + 1):
                            nc.tensor.matmul(
                                st_["qk"],
                                wqk_sb[:, c, :],
                                xt_tiles[i][:, c, :],
                                start=(c == 0),
                                stop=(c == 3),
                                skip_group_check=True,
                            )
                    return f

                def qk_ex():
                    qk_ps = st_["qk"]
                    cols = slice(tb * 512, (tb + 1) * 512)
                    nc.vector.tensor_scalar_add(
                        qt2[b][0:64, cols], qk_ps[0:64, :], bqk_sb[0:64, 0:1]
                    )
                    nc.sync.dma_start(
                        out=qt2[b][64:128, cols], in_=qt2[b][0:64, cols]
                    )
                    kts = ktspool.tile([128, 512], F32R, tag="kts")
                    nc.vector.tensor_scalar_add(
                        kts[64:128, :], qk_ps[64:128, :], bqk_sb[64:128, 0:1]
                    )
                    # pair-pack K.T: even k-tile -> rows 0:64, odd -> 64:128
                    kts_r = kts[64:128, :].rearrange(
                        "p (j1 j2 c) -> p j1 j2 c", j2=2, c=128
                    )
                    c0 = 256 * tb
                    nc.sync.dma_start(
                        out=kt2[b][0:64, c0 : c0 + 256].rearrange(
                            "p (j1 one c) -> p j1 one c", one=1, c=128
                        ),
                        in_=kts_r[:, :, 0:1, :],
                    )
                    nc.sync.dma_start(
                        out=kt2[b][64:128, c0 : c0 + 256].rearrange(
                            "p (j1 one c) -> p j1 one c", one=1, c=128
                        ),
                        in_=kts_r[:, :, 1:2, :],
                    )

                def vv(c0):
                    def f():
                        if c0 == 0:
                            st_["v"] = ps_misc.tile(
                                [128, 512], F32, tag="m", name="vps"
                            )
                        for c in (c0, c0 + 1):
                            nc.tensor.matmul(
                                st_["v"][0:HD, :],
                                wv_sb[:, c, :],
                                xt_tiles[i][:, c, :],
                                start=(c == 0),
                                stop=(c == 3),
                                skip_group_check=True,
                            )
                    return f

                def v_ex():
                    vt_sb = kstpool.tile([HD, 512], BF16, tag="vt")
                    st_["vt"] = vt_sb
                    nc.vector.tensor_scalar_add(
                        vt_sb, st_["v"][0:HD, :], bv_sb[:, 0:1]
                    )

                def tr(jj):
                    def f():
                        for j in (jj, jj + 1):
                            kt = tb * 4 + j
                            vtr_ps = ps_misc.tile([128, HD], BF16, tag="m")
                            nc.tensor.transpose(
                                vtr_ps, st_["vt"][:, j * 128 : (j + 1) * 128], identb
                            )
                            nc.vector.tensor_copy(
                                vp[b][:, kt * 65 : kt * 65 + 64], vtr_ps
                            )
                    return f

                return [qk(0), qk(2), qk_ex, vv(0), vv(2), v_ex, tr(0), tr(2)]

            def attn_qblock(b, qb, fq):
                """Attention for q-block qb; pops one filler per chunk."""
                q0 = qb * 512
                if fq:  # prev block's u_ps consumers must emit before realloc
                    fq.popleft()()
                u_ps = ps_u.tile([65, 512], F32, tag="u")
                n_chunks = 2 * (qb + 1)
                n_kt = 2 * n_chunks

                def emit_av(pt, j):
                    for j2 in range(2):
                        kt = 2 * j + j2
                        nc.tensor.matmul(
                            u_ps,
                            vp[b][:, kt * 65 : kt * 65 + 65],
                            pt[:, j2, :],
                            start=(kt == 0),
                            stop=(kt == n_kt - 1),
                            skip_group_check=True,
                        )

                prev_pt = None
                for j in range(n_chunks):
                    st = ps_st.tile([128, 2, 512], F32, tag="st")
                    nc.tensor.matmul(
                        st[:, 0, :],
                        kt2[b][0:64, j * 128 : (j + 1) * 128],
                        qt2[b][0:64, q0 : q0 + 512],
                        start=True,
                        stop=True,
                        tile_position=(0, 0),
                    )
                    nc.tensor.matmul(
                        st[:, 1, :],
                        kt2[b][64:128, j * 128 : (j + 1) * 128],
                        qt2[b][64:128, q0 : q0 + 512],
                        start=True,
                        stop=True,
                        tile_position=(64, 0),
                    )
                    pt = ptpool.tile([128, 2, 512], BF16, tag="pt")
                    nc.scalar.activation(
                        pt, st, mybir.ActivationFunctionType.Exp, scale=SCALE
                    )
                    if j >= n_chunks - 2:  # diagonal chunks: causal mask
                        d0 = (j % 2) * 2
                        # flat 2D APs so DVE picks the 2x packed-bf16 mode
                        nc.vector.tensor_mul(
                            pt.rearrange("p a b -> p (a b)"),
                            pt.rearrange("p a b -> p (a b)"),
                            mask_sb.rearrange("p a b -> p (a b)")[
                                :, d0 * 512 : (d0 + 2) * 512
                            ],
                        )
                    if prev_pt is not None:
                        emit_av(prev_pt, j - 1)
                    prev_pt = pt
                    if fq:
                        fq.popleft()()
                emit_av(prev_pt, n_chunks - 1)
                return u_ps

            def fin_pieces(b, qb, u_ps):
                """U->SBUF, L out, row-packed o_proj pairs, y out (bf16)."""
                row0 = b * S + qb * 512
                st_ = {}

                def p1():
                    u_sb = upool.tile([128, 512], F32R, tag="u")
                    st_["u"] = u_sb
                    nc.vector.tensor_copy(u_sb[0:64, :], u_ps[0:64, :])
                    nc.vector.tensor_copy(
                        l_acc[0:1, row0 : row0 + 512], u_ps[64:65, :]
                    )
                    nc.sync.dma_start(out=u_sb[64:128, :], in_=u_sb[0:64, :])

                def half(jp, k):
                    def f():
                        u_sb = st_["u"]
                        if k == 0:
                            st_[f"y{jp}"] = ypool.tile(
                                [128, 2, 512], BF16, tag="y", name=f"ysb{jp}"
                            )
                        ysb = st_[f"y{jp}"]
                        j2 = 2 * jp + k
                        y_ps = ps_misc.tile([128, 512], F32, tag="m", name="yps")
                        nc.tensor.matmul(
                            y_ps,
                            u_sb[k * 64 : (k + 1) * 64, j2 * 128 : (j2 + 1) * 128],
                            wo_sb[k * 64 : (k + 1) * 64, :],
                            start=True,
                            stop=True,
                            tile_position=(k * 64, 0),
                        )
                        nc.vector.tensor_copy(ysb[:, k, :], y_ps)
                        if k == 1:
                            r0 = row0 + jp * 256
                            nc.gpsimd.dma_start(
                                out=y_d.ap()[r0 : r0 + 256, :].rearrange(
                                    "(j p) d -> p j d", p=128
                                ),
                                in_=ysb,
                            )
                    return f

                return [p1, half(0, 0), half(0, 1), half(1, 0), half(1, 1)]

            # --- main pipeline -----------------------------------------
            fq = deque()
            for piece in proj_pieces(0):
                piece()
            vp_ones(0)
            fq.append(lambda: vp_ones(1))
            for i, (b, t) in enumerate(blocks):
                start_xt(i + 2)
                if i + 1 < len(blocks):
                    fq.extend(proj_pieces(i + 1))
                u_ps = attn_qblock(b, t, fq)
                while fq:
                    fq.popleft()()
                fq.extend(fin_pieces(b, t, u_ps))
            while fq:
                fq.popleft()()
            nc.sync.dma_start(
                out=l_d.ap().rearrange("(p c) -> p c", p=1), in_=l_acc
            )

    nc.compile()
    return nc


def _prep_inputs(x, Wq, bq, Wk, bk, Wv, bv, Wo, bo):
    import ml_dtypes

    xt = np.ascontiguousarray(x.reshape(TOK, D).T).astype(np.float32)
    mask = np.zeros((128, 4, 512), dtype=np.float32)
    p = np.arange(128)[:, None]
    c = np.arange(512)[None, :]
    for d in range(4):
        mask[:, d, :] = (p + 128 * d <= c).astype(np.float32)
    mask = mask.astype(ml_dtypes.bfloat16)
    identb = np.eye(64, dtype=np.float32).astype(ml_dtypes.bfloat16)
    onesb = np.ones((128, NKT), dtype=np.float32).astype(ml_dtypes.bfloat16)

    in_maps = []
    for h in range(H):
        hs = slice(h * HD, (h + 1) * HD)
        wo_h = np.ascontiguousarray(Wo[hs, :]).astype(np.float32)
        in_maps.append(
            {
                "xt": xt,
                "wqk": np.ascontiguousarray(
                    np.concatenate([Wq[:, hs], Wk[:, hs]], axis=1)
                ).astype(np.float32),
                "wv": np.ascontiguousarray(Wv[:, hs]).astype(np.float32),
                "wo": np.concatenate([wo_h, wo_h], axis=0),
                "bqk": np.concatenate([bq[hs], bk[hs]]).reshape(128, 1).astype(
                    np.float32
                ),
                "bv": bv[hs].reshape(HD, 1).astype(np.float32),
                "mask": mask,
                "identb": identb,
                "onesb": onesb,
            }
        )
    return in_maps


def _install_ntff_hook():
    """Register the axon NTFF profiling hook (test-only plumbing)."""
    import types

    try:
        from antenv.axon_hooks import set_axon_ntff_profile_hook  # noqa: F401
    except ImportError:
        m = types.ModuleType("antenv.axon_hooks")
        m._HOOK = None
        m.set_axon_ntff_profile_hook = lambda h: setattr(m, "_HOOK", h)
        m.get_axon_ntff_profile_hook = lambda: m._HOOK
        sys.modules["antenv.axon_hooks"] = m
        import antenv

        antenv.axon_hooks = m
    from antenv.axon_hooks import (
        get_axon_ntff_profile_hook,
        set_axon_ntff_profile_hook,
    )

    if get_axon_ntff_profile_hook() is None:
        import trn_agent_boot.trn_boot as tb

        set_axon_ntff_profile_hook(
            tb._ntff_profile_via_ctypes("/opt/axon/libaxon_pjrt.so")
        )


def kernel(x, Wq, bq, Wk, bk, Wv, bv, Wo, bo, _trace=False):
    x, Wq, bq, Wk, bk, Wv, bv, Wo, bo = (
        np.asarray(a, dtype=np.float32) for a in (x, Wq, bq, Wk, bk, Wv, bv, Wo, bo)
    )
    if "nc" not in _CACHE:
        _CACHE["nc"] = _build()
    nc = _CACHE["nc"]
    in_maps = _prep_inputs(x, Wq, bq, Wk, bk, Wv, bv, Wo, bo)
    kwargs = {}
    if _trace:
        _install_ntff_hook()
        kwargs = dict(trace=True, trace_cores=[0])
    res = run_bass_kernel_spmd(nc, in_maps, core_ids=list(range(8)), **kwargs)
    _CACHE["last_result"] = res
    y = np.zeros((TOK, D), dtype=np.float64)
    for r in res.results:
        y += r["y"].astype(np.float64) / r["l"].astype(np.float64)[:, None]
    y += bo[None, :]
    return y.astype(np.float32).reshape(B, S, D)


# revision 17
# speedup vs baseline: 1.1790x; 1.1790x over previous
"""Causal self-attention (B=2, S=4096, D=512, H=8) on 8 Trainium2 NeuronCores.

Sharding: tensor-parallel over heads. Core h computes head h for both batch
elements: QKV projections for its head, causal flash attention, and its
partial (unnormalized) o_proj contribution y_h = U_h @ Wo[h*64:(h+1)*64, :]
plus the per-query softmax denominators L_h. The host computes
sum_h(y_h / L_h) + bo.

Per-core structure (hd = 64, S = 4096, 32 k-tiles of 128 per batch):
  - xT [512, 8192] (host-pretransposed x) streams in as [128f, 4c, 512t]
    tiles; QK projection matmul (lhsT = [Wq_h | Wk_h] chunk) produces
    psum [Q.T; K.T] per 512-token block; V.T separately, then PE-transposed
    to V natural (bf16).
  - QT2 [128, 4096]/batch: Q.T duplicated in both partition halves (dup via
    SBUF->SBUF DMA). KT2 [128, 2048]/batch: K.T pair-packed -- even k-tiles
    in partitions 0:64, odd k-tiles in partitions 64:128.
  - Scores: per k-tile pair, TWO K=64 matmuls at tile_position (0,0) and
    (64,0) (disjoint PE row groups -> run concurrently): S.T chunk psum
    [128, 2, 512]. One ACT exp call [128, 1024] PSUM->SBUF produces P.T in
    bf16; diagonal chunks get a 0/1 causal mask multiply on DVE.
  - AV (bf16): U'[65, 512] += V'_kt.T @ P.T_kt with V' = [V | ones]; row 64
    accumulates L. AV for chunk j is emitted after the scores of chunk j+1.
  - o_proj: U dup'd to both partition halves; per q-subtile pair, TWO K=64
    matmuls at row groups 0/64 against Wo_h (host-duplicated into both
    halves); y out in bf16, 256 rows per DMA, unnormalized, plus L.
  - proj of block t+1 and o_proj of block t-1 are emitted as small "filler"
    pieces between attention chunks so the Scalar engine (exp, the
    throughput floor at ~1.15us per 1024-col chunk) never starves and the
    PE never idles long enough to re-throttle (HAM).

Matmuls: scores/projections/o_proj in float32r (~1.6e-4), AV in bf16.
"""

import sys

for _p in ("/opt/trn_rl_repo", "/root/.axon_site/_ro/trn_rl_repo"):
    if _p not in sys.path:
        sys.path.insert(0, _p)

from collections import deque

import numpy as np

import concourse.bass as bass
import concourse.mybir as mybir
import concourse.tile as tile
from concourse import bacc
from concourse.bass_utils import run_bass_kernel_spmd

B = 2
S = 4096
D = 512
H = 8
HD = 64
TOK = B * S          # 8192
NKT = S // 128       # 32 k-tiles per batch
SCALE = HD ** -0.5

F32 = mybir.dt.float32
F32R = mybir.dt.float32r
BF16 = mybir.dt.bfloat16

_CACHE = {}


def _build():
    nc = bacc.Bacc("TRN2", target_bir_lowering=False, debug=False, num_devices=8)

    xt_d = nc.dram_tensor("xt", [D, TOK], F32R, kind="ExternalInput")
    wqk_d = nc.dram_tensor("wqk", [D, 128], F32R, kind="ExternalInput")
    wv_d = nc.dram_tensor("wv", [D, HD], F32R, kind="ExternalInput")
    wo_d = nc.dram_tensor("wo", [128, D], F32R, kind="ExternalInput")
    bqk_d = nc.dram_tensor("bqk", [128, 1], F32, kind="ExternalInput")
    bv_d = nc.dram_tensor("bv", [HD, 1], F32, kind="ExternalInput")
    mask_d = nc.dram_tensor("mask", [128, 4, 512], BF16, kind="ExternalInput")
    identb_d = nc.dram_tensor("identb", [64, 64], BF16, kind="ExternalInput")
    onesb_d = nc.dram_tensor("onesb", [128, NKT], BF16, kind="ExternalInput")
    y_d = nc.dram_tensor("y", [TOK, D], BF16, kind="ExternalOutput")
    l_d = nc.dram_tensor("l", [TOK], F32R, kind="ExternalOutput")

    xt_r = xt_d.ap().rearrange("(c p) t -> p c t", p=128)      # [128, 4, 8192]
    wqk_r = wqk_d.ap().rearrange("(c p) m -> p c m", p=128)    # [128, 4, 128]
    wv_r = wv_d.ap().rearrange("(c p) m -> p c m", p=128)      # [128, 4, 64]

    with tile.TileContext(nc) as tc:
        import contextlib

        with contextlib.ExitStack() as ctx:
            singles = ctx.enter_context(tc.tile_pool(name="singles", bufs=1))
            xpool = ctx.enter_context(tc.tile_pool(name="xt", bufs=3))
            ptpool = ctx.enter_context(tc.tile_pool(name="pt", bufs=5))
            upool = ctx.enter_context(tc.tile_pool(name="usb", bufs=2))
            ypool = ctx.enter_context(tc.tile_pool(name="ysb", bufs=4))
            kstpool = ctx.enter_context(tc.tile_pool(name="kst", bufs=2))
            ktspool = ctx.enter_context(tc.tile_pool(name="kts", bufs=2))

            ps_st = ctx.enter_context(
                tc.tile_pool(name="ps_st", bufs=2, space="PSUM")
            )
            ps_u = ctx.enter_context(tc.tile_pool(name="ps_u", bufs=2, space="PSUM"))
            ps_misc = ctx.enter_context(
                tc.tile_pool(name="ps_misc", bufs=2, space="PSUM")
            )

            # --- persistent per-batch activation buffers ---------------
            qt2 = [
                singles.tile([128, S], F32R, tag=f"qt2_{b}", name=f"qt2_{b}")
                for b in range(B)
            ]
            kt2 = [
                singles.tile([128, S // 2], F32R, tag=f"kt2_{b}", name=f"kt2_{b}")
                for b in range(B)
            ]
            vp = [
                singles.tile([128, NKT * 65], BF16, tag=f"vp_{b}", name=f"vp_{b}")
                for b in range(B)
            ]
            l_acc = singles.tile([1, TOK], F32R, name="l_acc")

            blocks = [(b, t) for b in range(B) for t in range(8)]
            xt_tiles = {}

            def start_xt(i):
                if i >= len(blocks):
                    return
                b, tb = blocks[i]
                t0 = b * S + tb * 512
                xt_sb = xpool.tile([128, 4, 512], F32R, tag="xt")
                nc.gpsimd.dma_start(out=xt_sb, in_=xt_r[:, :, t0 : t0 + 512])
                xt_tiles[i] = xt_sb

            # --- constants / weights (xt(0) first: it gates proj(0)) ---
            start_xt(0)
            wqk_sb = singles.tile([128, 4, 128], F32R)
            wv_sb = singles.tile([128, 4, HD], F32R)
            wo_sb = singles.tile([128, D], F32R)
            bqk_sb = singles.tile([128, 1], F32)
            bv_sb = singles.tile([HD, 1], F32)
            mask_sb = singles.tile([128, 4, 512], BF16)
            identb = singles.tile([64, 64], BF16)
            ones_sb = singles.tile([128, NKT], BF16, name="ones_sb")
            nc.sync.dma_start(out=wqk_sb, in_=wqk_r)
            nc.sync.dma_start(out=bqk_sb, in_=bqk_d.ap())
            nc.sync.dma_start(out=wv_sb, in_=wv_r)
            nc.sync.dma_start(out=bv_sb, in_=bv_d.ap())
            nc.sync.dma_start(out=identb, in_=identb_d.ap())
            nc.sync.dma_start(out=ones_sb, in_=onesb_d.ap())
            nc.sync.dma_start(out=mask_sb, in_=mask_d.ap())
            nc.sync.dma_start(out=wo_sb, in_=wo_d.ap())
            start_xt(1)

            def vp_ones(b):
                nc.vector.tensor_copy(
                    vp[b].rearrange("p (t c) -> p t c", c=65)[:, :, 64:65],
                    ones_sb.rearrange("p (t c) -> p t c", c=1),
                )

            def proj_pieces(i):
                """Projections for block i as a list of small emit-closures."""
                b, tb = blocks[i]
                st_ = {}

                def qk(c0):
                    def f():
                        if c0 == 0:
                            st_["qk"] = ps_misc.tile(
                                [128, 512], F32, tag="m", name="qkps"
                            )
                        for c in (c0, c0 + 1):
                            nc.tensor.matmul(
                                st_["qk"],
                                wqk_sb[:, c, :],
                                xt_tiles[i][:, c, :],
                                start=(c == 0),
                                stop=(c == 3),
                                skip_group_check=True,
                            )
                    return f

                def qk_ex():
                    qk_ps = st_["qk"]
                    cols = slice(tb * 512, (tb + 1) * 512)
                    nc.vector.tensor_scalar_add(
                        qt2[b][0:64, cols], qk_ps[0:64, :], bqk_sb[0:64, 0:1]
                    )
                    nc.sync.dma_start(
                        out=qt2[b][64:128, cols], in_=qt2[b][0:64, cols]
                    )
                    kts = ktspool.tile([128, 512], F32R, tag="kts")
                    nc.vector.tensor_scalar_add(
                        kts[64:128, :], qk_ps[64:128, :], bqk_sb[64:128, 0:1]
                    )
                    # pair-pack K.T: even k-tile -> rows 0:64, odd -> 64:128
                    kts_r = kts[64:128, :].rearrange(
                        "p (j1 j2 c) -> p j1 j2 c", j2=2, c=128
                    )
                    c0 = 256 * tb
                    nc.sync.dma_start(
                        out=kt2[b][0:64, c0 : c0 + 256].rearrange(
                            "p (j1 one c) -> p j1 one c", one=1, c=128
                        ),
                        in_=kts_r[:, :, 0:1, :],
                    )
                    nc.sync.dma_start(
                        out=kt2[b][64:128, c0 : c0 + 256].rearrange(
                            "p (j1 one c) -> p j1 one c", one=1, c=128
                        ),
                        in_=kts_r[:, :, 1:2, :],
                    )

                def vv(c0):
                    def f():
                        if c0 == 0:
                            st_["v"] = ps_misc.tile(
                                [128, 512], F32, tag="m", name="vps"
                            )
                        for c in (c0, c0 + 1):
                            nc.tensor.matmul(
                                st_["v"][0:HD, :],
                                wv_sb[:, c, :],
                                xt_tiles[i][:, c, :],
                                start=(c == 0),
                                stop=(c == 3),
                                skip_group_check=True,
                            )
                    return f

                def v_ex():
                    vt_sb = kstpool.tile([HD, 512], BF16, tag="vt")
                    st_["vt"] = vt_sb
                    nc.vector.tensor_scalar_add(
                        vt_sb, st_["v"][0:HD, :], bv_sb[:, 0:1]
                    )

                def tr(jj):
                    def f():
                        for j in (jj, jj + 1):
                            kt = tb * 4 + j
                            vtr_ps = ps_misc.tile([128, HD], BF16, tag="m")
                            nc.tensor.transpose(
                                vtr_ps, st_["vt"][:, j * 128 : (j + 1) * 128], identb
                            )
                            nc.vector.tensor_copy(
                                vp[b][:, kt * 65 : kt * 65 + 64], vtr_ps
                            )
                    return f

                return [qk(0), qk(2), qk_ex, vv(0), vv(2), v_ex, tr(0), tr(2)]

            def attn_qblock(b, qb, fq):
                """Attention for q-block qb; pops one filler per chunk."""
                q0 = qb * 512
                if fq:  # prev block's u_ps consumers must emit before realloc
                    fq.popleft()()
                u_ps = ps_u.tile([65, 512], F32, tag="u")
                n_chunks = 2 * (qb + 1)
                n_kt = 2 * n_chunks

                def emit_av(pt, j):
                    for j2 in range(2):
                        kt = 2 * j + j2
                        nc.tensor.matmul(
                            u_ps,
                            vp[b][:, kt * 65 : kt * 65 + 65],
                            pt[:, j2, :],
                            start=(kt == 0),
                            stop=(kt == n_kt - 1),
                            skip_group_check=True,
                        )

                prev_pt = None
                for j in range(n_chunks):
                    st = ps_st.tile([128, 2, 512], F32, tag="st")
                    nc.tensor.matmul(
                        st[:, 0, :],
                        kt2[b][0:64, j * 128 : (j + 1) * 128],
                        qt2[b][0:64, q0 : q0 + 512],
                        start=True,
                        stop=True,
                        tile_position=(0, 0),
                    )
                    nc.tensor.matmul(
                        st[:, 1, :],
                        kt2[b][64:128, j * 128 : (j + 1) * 128],
                        qt2[b][64:128, q0 : q0 + 512],
                        start=True,
                        stop=True,
                        tile_position=(64, 0),
                    )
                    pt = ptpool.tile([128, 2, 512], BF16, tag="pt")
                    nc.scalar.activation(
                        pt, st, mybir.ActivationFunctionType.Exp, scale=SCALE
                    )
                    if j >= n_chunks - 2:  # diagonal chunks: causal mask
                        d0 = (j % 2) * 2
                        nc.vector.tensor_mul(pt, pt, mask_sb[:, d0 : d0 + 2, :])
                    if prev_pt is not None:
                        emit_av(prev_pt, j - 1)
                    prev_pt = pt
                    if fq:
                        fq.popleft()()
                emit_av(prev_pt, n_chunks - 1)
                return u_ps

            def fin_pieces(b, qb, u_ps):
                """U->SBUF, L out, row-packed o_proj pairs, y out (bf16)."""
                row0 = b * S + qb * 512
                st_ = {}

                def p1():
                    u_sb = upool.tile([128, 512], F32R, tag="u")
                    st_["u"] = u_sb
                    nc.vector.tensor_copy(u_sb[0:64, :], u_ps[0:64, :])
                    nc.vector.tensor_copy(
                        l_acc[0:1, row0 : row0 + 512], u_ps[64:65, :]
                    )
                    nc.sync.dma_start(out=u_sb[64:128, :], in_=u_sb[0:64, :])

                def half(jp, k):
                    def f():
                        u_sb = st_["u"]
                        if k == 0:
                            st_[f"y{jp}"] = ypool.tile(
                                [128, 2, 512], BF16, tag="y", name=f"ysb{jp}"
                            )
                        ysb = st_[f"y{jp}"]
                        j2 = 2 * jp + k
                        y_ps = ps_misc.tile([128, 512], F32, tag="m", name="yps")
                        nc.tensor.matmul(
                            y_ps,
                            u_sb[k * 64 : (k + 1) * 64, j2 * 128 : (j2 + 1) * 128],
                            wo_sb[k * 64 : (k + 1) * 64, :],
                            start=True,
                            stop=True,
                            tile_position=(k * 64, 0),
                        )
                        nc.vector.tensor_copy(ysb[:, k, :], y_ps)
                        if k == 1:
                            r0 = row0 + jp * 256
                            nc.gpsimd.dma_start(
                                out=y_d.ap()[r0 : r0 + 256, :].rearrange(
                                    "(j p) d -> p j d", p=128
                                ),
                                in_=ysb,
                            )
                    return f

                return [p1, half(0, 0), half(0, 1), half(1, 0), half(1, 1)]

            # --- main pipeline -----------------------------------------
            fq = deque()
            for piece in proj_pieces(0):
                piece()
            vp_ones(0)
            fq.append(lambda: vp_ones(1))
            for i, (b, t) in enumerate(blocks):
                start_xt(i + 2)
                if i + 1 < len(blocks):
                    fq.extend(proj_pieces(i + 1))
                u_ps = attn_qblock(b, t, fq)
                while fq:
                    fq.popleft()()
                fq.extend(fin_pieces(b, t, u_ps))
            while fq:
                fq.popleft()()
            nc.sync.dma_start(
                out=l_d.ap().rearrange("(p c) -> p c", p=1), in_=l_acc
            )

    nc.compile()
    return nc


def _prep_inputs(x, Wq, bq, Wk, bk, Wv, bv, Wo, bo):
    import ml_dtypes

    xt = np.ascontiguousarray(x.reshape(TOK, D).T).astype(np.float32)
    mask = np.zeros((128, 4, 512), dtype=np.float32)
    p = np.arange(128)[:, None]
    c = np.arange(512)[None, :]
    for d in range(4):
        mask[:, d, :] = (p + 128 * d <= c).astype(np.float32)
    mask = mask.astype(ml_dtypes.bfloat16)
    identb = np.eye(64, dtype=np.float32).astype(ml_dtypes.bfloat16)
    onesb = np.ones((128, NKT), dtype=np.float32).astype(ml_dtypes.bfloat16)

    in_maps = []
    for h in range(H):
        hs = slice(h * HD, (h + 1) * HD)
        wo_h = np.ascontiguousarray(Wo[hs, :]).astype(np.float32)
        in_maps.append(
            {
                "xt": xt,
                "wqk": np.ascontiguousarray(
                    np.concatenate([Wq[:, hs], Wk[:, hs]], axis=1)
                ).astype(np.float32),
                "wv": np.ascontiguousarray(Wv[:, hs]).astype(np.float32),
                "wo": np.concatenate([wo_h, wo_h], axis=0),
                "bqk": np.concatenate([bq[hs], bk[hs]]).reshape(128, 1).astype(
                    np.float32
                ),
                "bv": bv[hs].reshape(HD, 1).astype(np.float32),
                "mask": mask,
                "identb": identb,
                "onesb": onesb,
            }
        )
    return in_maps


def _install_ntff_hook():
    """Register the axon NTFF profiling hook (test-only plumbing)."""
    import types

    try:
        from antenv.axon_hooks import set_axon_ntff_profile_hook  # noqa: F401
    except ImportError:
        m = types.ModuleType("antenv.axon_hooks")
        m._HOOK = None
        m.set_axon_ntff_profile_hook = lambda h: setattr(m, "_HOOK", h)
        m.get_axon_ntff_profile_hook = lambda: m._HOOK
        sys.modules["antenv.axon_hooks"] = m
        import antenv

        antenv.axon_hooks = m
    from antenv.axon_hooks import (
        get_axon_ntff_profile_hook,
        set_axon_ntff_profile_hook,
    )

    if get_axon_ntff_profile_hook() is None:
        import trn_agent_boot.trn_boot as tb

        set_axon_ntff_profile_hook(
            tb._ntff_profile_via_ctypes("/opt/axon/libaxon_pjrt.so")
        )


def kernel(x, Wq, bq, Wk, bk, Wv, bv, Wo, bo, _trace=False):
    x, Wq, bq, Wk, bk, Wv, bv, Wo, bo = (
        np.asarray(a, dtype=np.float32) for a in (x, Wq, bq, Wk, bk, Wv, bv, Wo, bo)
    )
    if "nc" not in _CACHE:
        _CACHE["nc"] = _build()
    nc = _CACHE["nc"]
    in_maps = _prep_inputs(x, Wq, bq, Wk, bk, Wv, bv, Wo, bo)
    kwargs = {}
    if _trace:
        _install_ntff_hook()
        kwargs = dict(trace=True, trace_cores=[0])
    res = run_bass_kernel_spmd(nc, in_maps, core_ids=list(range(8)), **kwargs)
    _CACHE["last_result"] = res
    y = np.zeros((TOK, D), dtype=np.float64)
    for r in res.results:
        y += r["y"].astype(np.float64) / r["l"].astype(np.float64)[:, None]
    y += bo[None, :]
    return y.astype(np.float32).reshape(B, S, D)


# revision 22
# speedup vs baseline: 1.1958x; 1.0142x over previous
"""Causal self-attention (B=2, S=4096, D=512, H=8) on 8 Trainium2 NeuronCores.

Sharding: tensor-parallel over heads. Core h computes head h for both batch
elements: QKV projections for its head, causal flash attention, and its
partial (unnormalized) o_proj contribution y_h = U_h @ Wo[h*64:(h+1)*64, :]
plus the per-query softmax denominators L_h. The host computes
sum_h(y_h / L_h) + bo.

Per-core structure (hd = 64, S = 4096, 32 k-tiles of 128 per batch):
  - xT [512, 8192] (host-pretransposed x) streams in as [128f, 4c, 512t]
    tiles; QK projection matmul (lhsT = [Wq_h | Wk_h] chunk) produces
    psum [Q.T; K.T] per 512-token block; V.T separately, then PE-transposed
    to V natural (bf16).
  - QT2 [128, 4096]/batch: Q.T duplicated in both partition halves (dup via
    SBUF->SBUF DMA). KT2 [128, 2048]/batch: K.T pair-packed -- even k-tiles
    in partitions 0:64, odd k-tiles in partitions 64:128.
  - Scores: per k-tile pair, TWO K=64 matmuls at tile_position (0,0) and
    (64,0) (disjoint PE row groups -> run concurrently): S.T chunk psum
    [128, 2, 512]. One ACT exp call [128, 1024] PSUM->SBUF produces P.T in
    bf16; diagonal chunks get a 0/1 causal mask multiply on DVE.
  - AV (bf16): U'[65, 512] += V'_kt.T @ P.T_kt with V' = [V | ones]; row 64
    accumulates L. AV for chunk j is emitted after the scores of chunk j+1.
  - o_proj: U dup'd to both partition halves; per q-subtile pair, TWO K=64
    matmuls at row groups 0/64 against Wo_h (host-duplicated into both
    halves); y out in bf16, 256 rows per DMA, unnormalized, plus L.
  - proj of block t+1 and o_proj of block t-1 are emitted as small "filler"
    pieces between attention chunks so the Scalar engine (exp, the
    throughput floor at ~1.15us per 1024-col chunk) never starves and the
    PE never idles long enough to re-throttle (HAM).

Matmuls: scores/projections/o_proj in float32r (~1.6e-4), AV in bf16.
"""

import sys

for _p in ("/opt/trn_rl_repo", "/root/.axon_site/_ro/trn_rl_repo"):
    if _p not in sys.path:
        sys.path.insert(0, _p)

from collections import deque

import numpy as np

import concourse.bass as bass
import concourse.mybir as mybir
import concourse.tile as tile
from concourse import bacc
from concourse.bass_utils import run_bass_kernel_spmd

B = 2
S = 4096
D = 512
H = 8
HD = 64
TOK = B * S          # 8192
NKT = S // 128       # 32 k-tiles per batch
SCALE = HD ** -0.5

F32 = mybir.dt.float32
F32R = mybir.dt.float32r
BF16 = mybir.dt.bfloat16

_CACHE = {}


def _build():
    nc = bacc.Bacc("TRN2", target_bir_lowering=False, debug=False, num_devices=8)

    xt_d = nc.dram_tensor("xt", [D, TOK], F32R, kind="ExternalInput")
    # all fp32 weights packed: wqk[512] | wv[256] | wo[512] | bqk[1] | bv[1]
    wpack_d = nc.dram_tensor("wpack", [128, 1282], F32R, kind="ExternalInput")
    # all bf16 constants packed: mask[2048] | identb[64] (rows 0:64)
    bpack_d = nc.dram_tensor("bpack", [128, 2112], BF16, kind="ExternalInput")
    onesb_d = nc.dram_tensor("onesb", [128, NKT], BF16, kind="ExternalInput")
    y_d = nc.dram_tensor("y", [TOK, D], BF16, kind="ExternalOutput")
    l_d = nc.dram_tensor("l", [TOK], F32R, kind="ExternalOutput")

    xt_r = xt_d.ap().rearrange("(c p) t -> p c t", p=128)      # [128, 4, 8192]

    with tile.TileContext(nc) as tc:
        import contextlib

        with contextlib.ExitStack() as ctx:
            singles = ctx.enter_context(tc.tile_pool(name="singles", bufs=1))
            xpool = ctx.enter_context(tc.tile_pool(name="xt", bufs=3))
            ptpool = ctx.enter_context(tc.tile_pool(name="pt", bufs=5))
            upool = ctx.enter_context(tc.tile_pool(name="usb", bufs=2))
            ypool = ctx.enter_context(tc.tile_pool(name="ysb", bufs=4))
            kstpool = ctx.enter_context(tc.tile_pool(name="kst", bufs=2))
            ktspool = ctx.enter_context(tc.tile_pool(name="kts", bufs=2))

            ps_st = ctx.enter_context(
                tc.tile_pool(name="ps_st", bufs=2, space="PSUM")
            )
            ps_u = ctx.enter_context(tc.tile_pool(name="ps_u", bufs=2, space="PSUM"))
            ps_misc = ctx.enter_context(
                tc.tile_pool(name="ps_misc", bufs=2, space="PSUM")
            )

            # --- persistent per-batch activation buffers ---------------
            qt2 = [
                singles.tile([128, S], F32R, tag=f"qt2_{b}", name=f"qt2_{b}")
                for b in range(B)
            ]
            kt2 = [
                singles.tile([128, S // 2], F32R, tag=f"kt2_{b}", name=f"kt2_{b}")
                for b in range(B)
            ]
            vp = [
                singles.tile([128, NKT * 65], BF16, tag=f"vp_{b}", name=f"vp_{b}")
                for b in range(B)
            ]
            l_acc = singles.tile([1, TOK], F32R, name="l_acc")

            blocks = [(b, t) for b in range(B) for t in range(8)]
            xt_tiles = {}

            def start_xt(i):
                if i >= len(blocks):
                    return
                b, tb = blocks[i]
                t0 = b * S + tb * 512
                xt_sb = xpool.tile([128, 4, 512], F32R, tag="xt")
                nc.gpsimd.dma_start(out=xt_sb, in_=xt_r[:, :, t0 : t0 + 512])
                xt_tiles[i] = xt_sb

            # --- constants / weights (xt(0) first: it gates proj(0)) ---
            start_xt(0)
            wpack_sb = singles.tile([128, 1282], F32R, name="wpack_sb")
            bpack_sb = singles.tile([128, 2112], BF16, name="bpack_sb")
            ones_sb = singles.tile([128, NKT], BF16, name="ones_sb")
            nc.sync.dma_start(out=wpack_sb, in_=wpack_d.ap())
            nc.sync.dma_start(out=bpack_sb, in_=bpack_d.ap())
            nc.sync.dma_start(out=ones_sb, in_=onesb_d.ap())
            start_xt(1)

            def wqk_sb(c):  # [128, 128] lhsT chunk c of [Wq_h | Wk_h]
                return wpack_sb[:, c * 128 : (c + 1) * 128]

            def wv_sbc(c):  # [128, 64] lhsT chunk c of Wv_h
                return wpack_sb[:, 512 + c * 64 : 512 + (c + 1) * 64]

            wo_sb = wpack_sb[:, 768:1280]      # [128, 512] Wo_h dup'd rows
            bqk_sb = wpack_sb[:, 1280:1281].bitcast(F32)    # [128, 1]
            bv_sb = wpack_sb[0:64, 1281:1282].bitcast(F32)  # [64, 1]
            identb = bpack_sb[0:64, 2048:2112]

            def vp_ones(b):
                nc.vector.tensor_copy(
                    vp[b].rearrange("p (t c) -> p t c", c=65)[:, :, 64:65],
                    ones_sb.rearrange("p (t c) -> p t c", c=1),
                )

            def proj_pieces(i):
                """Projections for block i as a list of small emit-closures."""
                b, tb = blocks[i]
                st_ = {}

                def qk(c0):
                    def f():
                        if c0 == 0:
                            st_["qk"] = ps_misc.tile(
                                [128, 512], F32, tag="m", name="qkps"
                            )
                        for c in (c0, c0 + 1):
                            nc.tensor.matmul(
                                st_["qk"],
                                wqk_sb(c),
                                xt_tiles[i][:, c, :],
                                start=(c == 0),
                                stop=(c == 3),
                                skip_group_check=True,
                            )
                    return f

                def qk_ex():
                    qk_ps = st_["qk"]
                    cols = slice(tb * 512, (tb + 1) * 512)
                    nc.vector.tensor_scalar_add(
                        qt2[b][0:64, cols], qk_ps[0:64, :], bqk_sb[0:64, 0:1]
                    )
                    nc.sync.dma_start(
                        out=qt2[b][64:128, cols], in_=qt2[b][0:64, cols]
                    )
                    kts = ktspool.tile([128, 512], F32R, tag="kts")
                    nc.vector.tensor_scalar_add(
                        kts[64:128, :], qk_ps[64:128, :], bqk_sb[64:128, 0:1]
                    )
                    # pair-pack K.T: even k-tile -> rows 0:64, odd -> 64:128
                    kts_r = kts[64:128, :].rearrange(
                        "p (j1 j2 c) -> p j1 j2 c", j2=2, c=128
                    )
                    c0 = 256 * tb
                    nc.sync.dma_start(
                        out=kt2[b][0:64, c0 : c0 + 256].rearrange(
                            "p (j1 one c) -> p j1 one c", one=1, c=128
                        ),
                        in_=kts_r[:, :, 0:1, :],
                    )
                    nc.sync.dma_start(
                        out=kt2[b][64:128, c0 : c0 + 256].rearrange(
                            "p (j1 one c) -> p j1 one c", one=1, c=128
                        ),
                        in_=kts_r[:, :, 1:2, :],
                    )

                def vv(c0):
                    def f():
                        if c0 == 0:
                            st_["v"] = ps_misc.tile(
                                [128, 512], F32, tag="m", name="vps"
                            )
                        for c in (c0, c0 + 1):
                            nc.tensor.matmul(
                                st_["v"][0:HD, :],
                                wv_sbc(c),
                                xt_tiles[i][:, c, :],
                                start=(c == 0),
                                stop=(c == 3),
                                skip_group_check=True,
                            )
                    return f

                def v_ex():
                    vt_sb = kstpool.tile([HD, 512], BF16, tag="vt")
                    st_["vt"] = vt_sb
                    nc.vector.tensor_scalar_add(
                        vt_sb, st_["v"][0:HD, :], bv_sb[:, 0:1]
                    )

                def tr(jj):
                    def f():
                        for j in (jj, jj + 1):
                            kt = tb * 4 + j
                            vtr_ps = ps_misc.tile([128, HD], BF16, tag="m")
                            nc.tensor.transpose(
                                vtr_ps, st_["vt"][:, j * 128 : (j + 1) * 128], identb
                            )
                            nc.vector.tensor_copy(
                                vp[b][:, kt * 65 : kt * 65 + 64], vtr_ps
                            )
                    return f

                return [qk(0), qk(2), qk_ex, vv(0), vv(2), v_ex, tr(0), tr(2)]

            def attn_qblock(b, qb, fq):
                """Attention for q-block qb; pops one filler per chunk."""
                q0 = qb * 512
                if fq:  # prev block's u_ps consumers must emit before realloc
                    fq.popleft()()
                u_ps = ps_u.tile([65, 512], F32, tag="u")
                n_chunks = 2 * (qb + 1)
                n_kt = 2 * n_chunks

                def emit_av(pt, j):
                    for j2 in range(2):
                        kt = 2 * j + j2
                        nc.tensor.matmul(
                            u_ps,
                            vp[b][:, kt * 65 : kt * 65 + 65],
                            pt[:, j2, :],
                            start=(kt == 0),
                            stop=(kt == n_kt - 1),
                            skip_group_check=True,
                        )

                prev_pt = None
                for j in range(n_chunks):
                    st = ps_st.tile([128, 2, 512], F32, tag="st")
                    nc.tensor.matmul(
                        st[:, 0, :],
                        kt2[b][0:64, j * 128 : (j + 1) * 128],
                        qt2[b][0:64, q0 : q0 + 512],
                        start=True,
                        stop=True,
                        tile_position=(0, 0),
                    )
                    nc.tensor.matmul(
                        st[:, 1, :],
                        kt2[b][64:128, j * 128 : (j + 1) * 128],
                        qt2[b][64:128, q0 : q0 + 512],
                        start=True,
                        stop=True,
                        tile_position=(64, 0),
                    )
                    pt = ptpool.tile([128, 2, 512], BF16, tag="pt")
                    nc.scalar.activation(
                        pt, st, mybir.ActivationFunctionType.Exp, scale=SCALE
                    )
                    if j >= n_chunks - 2:  # diagonal chunks: causal mask
                        d0 = (j % 2) * 2
                        ptf = pt.rearrange("p a b -> p (a b)")
                        nc.vector.tensor_mul(
                            ptf, ptf, bpack_sb[:, d0 * 512 : (d0 + 2) * 512]
                        )
                    if prev_pt is not None:
                        emit_av(prev_pt, j - 1)
                    prev_pt = pt
                    if fq:
                        fq.popleft()()
                emit_av(prev_pt, n_chunks - 1)
                return u_ps

            def fin_pieces(b, qb, u_ps):
                """U->SBUF, L out, row-packed o_proj pairs, y out (bf16)."""
                row0 = b * S + qb * 512
                st_ = {}

                def p1():
                    u_sb = upool.tile([128, 512], F32R, tag="u")
                    st_["u"] = u_sb
                    nc.vector.tensor_copy(u_sb[0:64, :], u_ps[0:64, :])
                    nc.vector.tensor_copy(
                        l_acc[0:1, row0 : row0 + 512], u_ps[64:65, :]
                    )
                    nc.sync.dma_start(out=u_sb[64:128, :], in_=u_sb[0:64, :])

                def half(jp, k):
                    def f():
                        u_sb = st_["u"]
                        if k == 0:
                            st_[f"y{jp}"] = ypool.tile(
                                [128, 2, 512], BF16, tag="y", name=f"ysb{jp}"
                            )
                        ysb = st_[f"y{jp}"]
                        j2 = 2 * jp + k
                        y_ps = ps_misc.tile([128, 512], F32, tag="m", name="yps")
                        nc.tensor.matmul(
                            y_ps,
                            u_sb[k * 64 : (k + 1) * 64, j2 * 128 : (j2 + 1) * 128],
                            wo_sb[k * 64 : (k + 1) * 64, :],
                            start=True,
                            stop=True,
                            tile_position=(k * 64, 0),
                        )
                        nc.vector.tensor_copy(ysb[:, k, :], y_ps)
                        if k == 1:
                            r0 = row0 + jp * 256
                            nc.gpsimd.dma_start(
                                out=y_d.ap()[r0 : r0 + 256, :].rearrange(
                                    "(j p) d -> p j d", p=128
                                ),
                                in_=ysb,
                            )
                    return f

                return [p1, half(0, 0), half(0, 1), half(1, 0), half(1, 1)]

            # --- main pipeline -----------------------------------------
            fq = deque()
            for piece in proj_pieces(0):
                piece()
            vp_ones(0)
            fq.append(lambda: vp_ones(1))
            for i, (b, t) in enumerate(blocks):
                start_xt(i + 2)
                if i + 1 < len(blocks):
                    fq.extend(proj_pieces(i + 1))
                u_ps = attn_qblock(b, t, fq)
                while fq:
                    fq.popleft()()
                fq.extend(fin_pieces(b, t, u_ps))
            while fq:
                fq.popleft()()
            nc.sync.dma_start(
                out=l_d.ap().rearrange("(p c) -> p c", p=1), in_=l_acc
            )

    nc.compile()
    return nc


def _prep_inputs(x, Wq, bq, Wk, bk, Wv, bv, Wo, bo):
    import ml_dtypes

    xt = np.ascontiguousarray(x.reshape(TOK, D).T).astype(np.float32)
    mask = np.zeros((128, 4, 512), dtype=np.float32)
    p = np.arange(128)[:, None]
    c = np.arange(512)[None, :]
    for d in range(4):
        mask[:, d, :] = (p + 128 * d <= c).astype(np.float32)
    mask = mask.astype(ml_dtypes.bfloat16)
    identb = np.eye(64, dtype=np.float32).astype(ml_dtypes.bfloat16)
    onesb = np.ones((128, NKT), dtype=np.float32).astype(ml_dtypes.bfloat16)

    bpack = np.concatenate(
        [
            mask.astype(np.float32).reshape(128, 2048),
            np.concatenate([identb.astype(np.float32), np.zeros((64, 64), np.float32)], axis=0),
        ],
        axis=1,
    ).astype(ml_dtypes.bfloat16)

    in_maps = []
    for h in range(H):
        hs = slice(h * HD, (h + 1) * HD)
        wo_h = np.ascontiguousarray(Wo[hs, :]).astype(np.float32)
        wqk = np.concatenate([Wq[:, hs], Wk[:, hs]], axis=1).astype(np.float32)
        wv_h = np.asarray(Wv[:, hs], dtype=np.float32)
        wpack = np.concatenate(
            [
                wqk.reshape(4, 128, 128).transpose(1, 0, 2).reshape(128, 512),
                wv_h.reshape(4, 128, HD).transpose(1, 0, 2).reshape(128, 256),
                np.concatenate([wo_h, wo_h], axis=0),
                np.concatenate([bq[hs], bk[hs]]).reshape(128, 1).astype(np.float32),
                np.concatenate(
                    [bv[hs].astype(np.float32), np.zeros(HD, np.float32)]
                ).reshape(128, 1),
            ],
            axis=1,
        ).astype(np.float32)
        in_maps.append(
            {
                "xt": xt,
                "wpack": np.ascontiguousarray(wpack),
                "bpack": bpack,
                "onesb": onesb,
            }
        )
    return in_maps


def _install_ntff_hook():
    """Register the axon NTFF profiling hook (test-only plumbing)."""
    import types

    try:
        from antenv.axon_hooks import set_axon_ntff_profile_hook  # noqa: F401
    except ImportError:
        m = types.ModuleType("antenv.axon_hooks")
        m._HOOK = None
        m.set_axon_ntff_profile_hook = lambda h: setattr(m, "_HOOK", h)
        m.get_axon_ntff_profile_hook = lambda: m._HOOK
        sys.modules["antenv.axon_hooks"] = m
        import antenv

        antenv.axon_hooks = m
    from antenv.axon_hooks import (
        get_axon_ntff_profile_hook,
        set_axon_ntff_profile_hook,
    )

    if get_axon_ntff_profile_hook() is None:
        import trn_agent_boot.trn_boot as tb

        set_axon_ntff_profile_hook(
            tb._ntff_profile_via_ctypes("/opt/axon/libaxon_pjrt.so")
        )


def kernel(x, Wq, bq, Wk, bk, Wv, bv, Wo, bo, _trace=False):
    x, Wq, bq, Wk, bk, Wv, bv, Wo, bo = (
        np.asarray(a, dtype=np.float32) for a in (x, Wq, bq, Wk, bk, Wv, bv, Wo, bo)
    )
    if "nc" not in _CACHE:
        _CACHE["nc"] = _build()
    nc = _CACHE["nc"]
    in_maps = _prep_inputs(x, Wq, bq, Wk, bk, Wv, bv, Wo, bo)
    kwargs = {}
    if _trace:
        _install_ntff_hook()
        kwargs = dict(trace=True, trace_cores=[0])
    res = run_bass_kernel_spmd(nc, in_maps, core_ids=list(range(8)), **kwargs)
    _CACHE["last_result"] = res
    y = np.zeros((TOK, D), dtype=np.float64)
    for r in res.results:
        y += r["y"].astype(np.float64) / r["l"].astype(np.float64)[:, None]
    y += bo[None, :]
    return y.astype(np.float32).reshape(B, S, D)


# revision 24
# speedup vs baseline: 1.4132x; 1.1818x over previous
"""Causal self-attention (B=2, S=4096, D=512, H=8) on 8 Trainium2 NeuronCores.

Sharding: tensor-parallel over heads. Core h computes head h for both batch
elements: QKV projections for its head, causal flash attention, and its
partial (unnormalized) o_proj contribution y_h = U_h @ Wo[h*64:(h+1)*64, :]
plus the per-query softmax denominators L_h. The host computes
sum_h(y_h / L_h) + bo.

Per-core structure (hd = 64, S = 4096, 32 k-tiles of 128 per batch):
  - xT [512, 8192] (host-pretransposed x) streams in as [128f, 4c, 512t]
    tiles; QK projection matmul (lhsT = [Wq_h | Wk_h] chunk) produces
    psum [Q.T; K.T] per 512-token block; V.T separately, then PE-transposed
    to V natural (bf16).
  - QT2 [128, 4096]/batch: Q.T duplicated in both partition halves (dup via
    SBUF->SBUF DMA). KT2 [128, 2048]/batch: K.T pair-packed -- even k-tiles
    in partitions 0:64, odd k-tiles in partitions 64:128.
  - Scores: per k-tile pair, TWO K=64 matmuls at tile_position (0,0) and
    (64,0) (disjoint PE row groups -> run concurrently): S.T chunk psum
    [128, 2, 512]. One ACT exp call [128, 1024] PSUM->SBUF produces P.T in
    bf16; diagonal chunks get a 0/1 causal mask multiply on DVE.
  - AV (bf16): U'[65, 512] += V'_kt.T @ P.T_kt with V' = [V | ones]; row 64
    accumulates L. AV for chunk j is emitted after the scores of chunk j+1.
  - o_proj: U dup'd to both partition halves; per q-subtile pair, TWO K=64
    matmuls at row groups 0/64 against Wo_h (host-duplicated into both
    halves); y out in bf16, 256 rows per DMA, unnormalized, plus L.
  - proj of block t+1 and o_proj of block t-1 are emitted as small "filler"
    pieces between attention chunks so the Scalar engine (exp, the
    throughput floor at ~1.15us per 1024-col chunk) never starves and the
    PE never idles long enough to re-throttle (HAM).

Matmuls: scores/projections/o_proj in float32r (~1.6e-4), AV in bf16.
"""

import sys

for _p in ("/opt/trn_rl_repo", "/root/.axon_site/_ro/trn_rl_repo"):
    if _p not in sys.path:
        sys.path.insert(0, _p)

from collections import deque

import numpy as np

import concourse.bass as bass
import concourse.mybir as mybir
import concourse.tile as tile
from concourse import bacc
from concourse.bass_utils import run_bass_kernel_spmd

B = 2
S = 4096
D = 512
H = 8
HD = 64
TOK = B * S          # 8192
NKT = S // 128       # 32 k-tiles per batch
SCALE = HD ** -0.5

F32 = mybir.dt.float32
F32R = mybir.dt.float32r
BF16 = mybir.dt.bfloat16

_CACHE = {}


def _build():
    nc = bacc.Bacc("TRN2", target_bir_lowering=False, debug=False, num_devices=8)

    xt_d = nc.dram_tensor("xt", [D, TOK], F32R, kind="ExternalInput")
    # all fp32 weights packed: wqk[512] | wv[256] | wo[512] | bqk[1] | bv[1]
    wpack_d = nc.dram_tensor("wpack", [128, 1282], F32R, kind="ExternalInput")
    # all bf16 constants packed: mask[2048] | identb[64] (rows 0:64)
    bpack_d = nc.dram_tensor("bpack", [128, 2112], BF16, kind="ExternalInput")
    onesb_d = nc.dram_tensor("onesb", [128, NKT], BF16, kind="ExternalInput")
    y_d = nc.dram_tensor("y", [TOK, D], BF16, kind="ExternalOutput")
    l_d = nc.dram_tensor("l", [TOK], F32R, kind="ExternalOutput")

    xt_r = xt_d.ap().rearrange("(c p) t -> p c t", p=128)      # [128, 4, 8192]

    with tile.TileContext(nc) as tc:
        import contextlib

        with contextlib.ExitStack() as ctx:
            singles = ctx.enter_context(tc.tile_pool(name="singles", bufs=1))
            xpool = ctx.enter_context(tc.tile_pool(name="xt", bufs=4))
            ptpool = ctx.enter_context(tc.tile_pool(name="pt", bufs=5))
            upool = ctx.enter_context(tc.tile_pool(name="usb", bufs=2))
            ypool = ctx.enter_context(tc.tile_pool(name="ysb", bufs=6))
            kstpool = ctx.enter_context(tc.tile_pool(name="kst", bufs=2))
            ktspool = ctx.enter_context(tc.tile_pool(name="kts", bufs=2))

            ps_st = ctx.enter_context(
                tc.tile_pool(name="ps_st", bufs=2, space="PSUM")
            )
            ps_u = ctx.enter_context(tc.tile_pool(name="ps_u", bufs=2, space="PSUM"))
            ps_misc = ctx.enter_context(
                tc.tile_pool(name="ps_misc", bufs=2, space="PSUM")
            )

            # --- persistent per-batch activation buffers ---------------
            qt2 = [
                singles.tile([128, S], F32R, tag=f"qt2_{b}", name=f"qt2_{b}")
                for b in range(B)
            ]
            kt2 = [
                singles.tile([128, S // 2], F32R, tag=f"kt2_{b}", name=f"kt2_{b}")
                for b in range(B)
            ]
            vp = [
                singles.tile([128, NKT * 65], BF16, tag=f"vp_{b}", name=f"vp_{b}")
                for b in range(B)
            ]
            l_acc = singles.tile([1, TOK], F32R, name="l_acc")

            blocks = [(b, t) for b in range(B) for t in range(8)]
            xt_tiles = {}

            def start_xt(i):
                if i >= len(blocks):
                    return
                b, tb = blocks[i]
                t0 = b * S + tb * 512
                xt_sb = xpool.tile([128, 4, 512], F32R, tag="xt")
                nc.gpsimd.dma_start(out=xt_sb, in_=xt_r[:, :, t0 : t0 + 512])
                xt_tiles[i] = xt_sb

            # --- constants / weights (xt(0) first: it gates proj(0)) ---
            start_xt(0)
            wpack_sb = singles.tile([128, 1282], F32R, name="wpack_sb")
            bpack_sb = singles.tile([128, 2112], BF16, name="bpack_sb")
            ones_sb = singles.tile([128, NKT], BF16, name="ones_sb")
            nc.sync.dma_start(out=wpack_sb, in_=wpack_d.ap())
            nc.sync.dma_start(out=bpack_sb, in_=bpack_d.ap())
            nc.sync.dma_start(out=ones_sb, in_=onesb_d.ap())
            start_xt(1)

            def wqk_sb(c):  # [128, 128] lhsT chunk c of [Wq_h | Wk_h]
                return wpack_sb[:, c * 128 : (c + 1) * 128]

            def wv_sbc(c):  # [128, 64] lhsT chunk c of Wv_h
                return wpack_sb[:, 512 + c * 64 : 512 + (c + 1) * 64]

            wo_sb = wpack_sb[:, 768:1280]      # [128, 512] Wo_h dup'd rows
            bqk_sb = wpack_sb[:, 1280:1281].bitcast(F32)    # [128, 1]
            bv_sb = wpack_sb[0:64, 1281:1282].bitcast(F32)  # [64, 1]
            identb = bpack_sb[0:64, 2048:2112]

            def vp_ones(b):
                nc.vector.tensor_copy(
                    vp[b].rearrange("p (t c) -> p t c", c=65)[:, :, 64:65],
                    ones_sb.rearrange("p (t c) -> p t c", c=1),
                )

            def proj_pieces(i):
                """Projections for block i as a list of small emit-closures."""
                b, tb = blocks[i]
                st_ = {}

                def qk(c0):
                    def f():
                        if c0 == 0:
                            st_["qk"] = ps_misc.tile(
                                [128, 512], F32, tag="m", name="qkps"
                            )
                        for c in (c0, c0 + 1):
                            nc.tensor.matmul(
                                st_["qk"],
                                wqk_sb(c),
                                xt_tiles[i][:, c, :],
                                start=(c == 0),
                                stop=(c == 3),
                                skip_group_check=True,
                            )
                    return f

                def qk_ex():
                    qk_ps = st_["qk"]
                    cols = slice(tb * 512, (tb + 1) * 512)
                    nc.vector.tensor_scalar_add(
                        qt2[b][0:64, cols], qk_ps[0:64, :], bqk_sb[0:64, 0:1]
                    )
                    nc.sync.dma_start(
                        out=qt2[b][64:128, cols], in_=qt2[b][0:64, cols]
                    )
                    kts = ktspool.tile([128, 512], F32R, tag="kts")
                    nc.vector.tensor_scalar_add(
                        kts[64:128, :], qk_ps[64:128, :], bqk_sb[64:128, 0:1]
                    )
                    # pair-pack K.T: even k-tile -> rows 0:64, odd -> 64:128
                    kts_r = kts[64:128, :].rearrange(
                        "p (j1 j2 c) -> p j1 j2 c", j2=2, c=128
                    )
                    c0 = 256 * tb
                    nc.sync.dma_start(
                        out=kt2[b][0:64, c0 : c0 + 256].rearrange(
                            "p (j1 one c) -> p j1 one c", one=1, c=128
                        ),
                        in_=kts_r[:, :, 0:1, :],
                    )
                    nc.sync.dma_start(
                        out=kt2[b][64:128, c0 : c0 + 256].rearrange(
                            "p (j1 one c) -> p j1 one c", one=1, c=128
                        ),
                        in_=kts_r[:, :, 1:2, :],
                    )

                def vv(c0):
                    def f():
                        if c0 == 0:
                            st_["v"] = ps_misc.tile(
                                [128, 512], F32, tag="m", name="vps"
                            )
                        for c in (c0, c0 + 1):
                            nc.tensor.matmul(
                                st_["v"][0:HD, :],
                                wv_sbc(c),
                                xt_tiles[i][:, c, :],
                                start=(c == 0),
                                stop=(c == 3),
                                skip_group_check=True,
                            )
                    return f

                def v_ex():
                    vt_sb = kstpool.tile([HD, 512], BF16, tag="vt")
                    st_["vt"] = vt_sb
                    nc.vector.tensor_scalar_add(
                        vt_sb, st_["v"][0:HD, :], bv_sb[:, 0:1]
                    )

                def tr(jj):
                    def f():
                        for j in (jj, jj + 1):
                            kt = tb * 4 + j
                            vtr_ps = ps_misc.tile([128, HD], BF16, tag="m")
                            nc.tensor.transpose(
                                vtr_ps, st_["vt"][:, j * 128 : (j + 1) * 128], identb
                            )
                            nc.vector.tensor_copy(
                                vp[b][:, kt * 65 : kt * 65 + 64], vtr_ps
                            )
                    return f

                return [qk(0), qk(2), qk_ex, vv(0), vv(2), v_ex, tr(0), tr(2)]

            def attn_qblock(b, qb, fq):
                """Attention for q-block qb; pops one filler per chunk."""
                q0 = qb * 512
                if fq:  # prev block's u_ps consumers must emit before realloc
                    fq.popleft()[1]()
                u_ps = ps_u.tile([65, 512], F32, tag="u")
                n_chunks = 2 * (qb + 1)
                n_kt = 2 * n_chunks

                def emit_av(pt, j):
                    for j2 in range(2):
                        kt = 2 * j + j2
                        nc.tensor.matmul(
                            u_ps,
                            vp[b][:, kt * 65 : kt * 65 + 65],
                            pt[:, j2, :],
                            start=(kt == 0),
                            stop=(kt == n_kt - 1),
                            skip_group_check=True,
                        )

                prev_pt = None
                for j in range(n_chunks):
                    st = ps_st.tile([128, 2, 512], F32, tag="st")
                    nc.tensor.matmul(
                        st[:, 0, :],
                        kt2[b][0:64, j * 128 : (j + 1) * 128],
                        qt2[b][0:64, q0 : q0 + 512],
                        start=True,
                        stop=True,
                        tile_position=(0, 0),
                    )
                    nc.tensor.matmul(
                        st[:, 1, :],
                        kt2[b][64:128, j * 128 : (j + 1) * 128],
                        qt2[b][64:128, q0 : q0 + 512],
                        start=True,
                        stop=True,
                        tile_position=(64, 0),
                    )
                    pt = ptpool.tile([128, 2, 512], BF16, tag="pt")
                    nc.scalar.activation(
                        pt, st, mybir.ActivationFunctionType.Exp, scale=SCALE
                    )
                    if j >= n_chunks - 2:  # diagonal chunks: causal mask
                        d0 = (j % 2) * 2
                        ptf = pt.rearrange("p a b -> p (a b)")
                        nc.vector.tensor_mul(
                            ptf, ptf, bpack_sb[:, d0 * 512 : (d0 + 2) * 512]
                        )
                    if prev_pt is not None:
                        emit_av(prev_pt, j - 1)
                    prev_pt = pt
                    if fq:
                        fq.popleft()[1]()
                emit_av(prev_pt, n_chunks - 1)
                return u_ps

            def fin_pieces(b, qb, u_ps):
                """U->SBUF, L out, row-packed o_proj pairs, y out (bf16)."""
                row0 = b * S + qb * 512
                st_ = {}

                def p1():
                    u_sb = upool.tile([128, 512], F32R, tag="u")
                    st_["u"] = u_sb
                    nc.vector.tensor_copy(u_sb[0:64, :], u_ps[0:64, :])
                    nc.vector.tensor_copy(
                        l_acc[0:1, row0 : row0 + 512], u_ps[64:65, :]
                    )
                    nc.sync.dma_start(out=u_sb[64:128, :], in_=u_sb[0:64, :])

                def half(jp, k):
                    def f():
                        u_sb = st_["u"]
                        if k == 0:
                            st_[f"y{jp}"] = ypool.tile(
                                [128, 2, 512], BF16, tag="y", name=f"ysb{jp}"
                            )
                        ysb = st_[f"y{jp}"]
                        j2 = 2 * jp + k
                        y_ps = ps_misc.tile([128, 512], F32, tag="m", name="yps")
                        nc.tensor.matmul(
                            y_ps,
                            u_sb[k * 64 : (k + 1) * 64, j2 * 128 : (j2 + 1) * 128],
                            wo_sb[k * 64 : (k + 1) * 64, :],
                            start=True,
                            stop=True,
                            tile_position=(k * 64, 0),
                        )
                        nc.vector.tensor_copy(ysb[:, k, :], y_ps)
                        if k == 1:
                            r0 = row0 + jp * 256
                            nc.gpsimd.dma_start(
                                out=y_d.ap()[r0 : r0 + 256, :].rearrange(
                                    "(j p) d -> p j d", p=128
                                ),
                                in_=ysb,
                            )
                    return f

                return [p1, half(0, 0), half(0, 1), half(1, 0), half(1, 1)]

            # --- main pipeline -----------------------------------------
            # filler items are (deadline_iter, fn): a piece must be emitted
            # by the END of its deadline iteration; until then it spreads
            # into chunk slack (one pop per attention chunk).
            fq = deque()
            for piece in proj_pieces(0):
                piece()
            vp_ones(0)
            start_xt(2)
            fq.append((0, lambda: vp_ones(1)))
            fq.extend((0, p) for p in proj_pieces(1))
            for i, (b, t) in enumerate(blocks):
                start_xt(i + 3)
                if i + 2 < len(blocks):
                    fq.extend((i + 1, p) for p in proj_pieces(i + 2))
                u_ps = attn_qblock(b, t, fq)
                while fq and fq[0][0] <= i:
                    fq.popleft()[1]()
                fq.extend((i + 1, p) for p in fin_pieces(b, t, u_ps))
            while fq:
                fq.popleft()[1]()
            nc.sync.dma_start(
                out=l_d.ap().rearrange("(p c) -> p c", p=1), in_=l_acc
            )

    nc.compile()
    return nc


def _prep_inputs(x, Wq, bq, Wk, bk, Wv, bv, Wo, bo):
    import ml_dtypes

    xt = np.ascontiguousarray(x.reshape(TOK, D).T).astype(np.float32)
    mask = np.zeros((128, 4, 512), dtype=np.float32)
    p = np.arange(128)[:, None]
    c = np.arange(512)[None, :]
    for d in range(4):
        mask[:, d, :] = (p + 128 * d <= c).astype(np.float32)
    mask = mask.astype(ml_dtypes.bfloat16)
    identb = np.eye(64, dtype=np.float32).astype(ml_dtypes.bfloat16)
    onesb = np.ones((128, NKT), dtype=np.float32).astype(ml_dtypes.bfloat16)

    bpack = np.concatenate(
        [
            mask.astype(np.float32).reshape(128, 2048),
            np.concatenate([identb.astype(np.float32), np.zeros((64, 64), np.float32)], axis=0),
        ],
        axis=1,
    ).astype(ml_dtypes.bfloat16)

    in_maps = []
    for h in range(H):
        hs = slice(h * HD, (h + 1) * HD)
        wo_h = np.ascontiguousarray(Wo[hs, :]).astype(np.float32)
        wqk = np.concatenate([Wq[:, hs], Wk[:, hs]], axis=1).astype(np.float32)
        wv_h = np.asarray(Wv[:, hs], dtype=np.float32)
        wpack = np.concatenate(
            [
                wqk.reshape(4, 128, 128).transpose(1, 0, 2).reshape(128, 512),
                wv_h.reshape(4, 128, HD).transpose(1, 0, 2).reshape(128, 256),
                np.concatenate([wo_h, wo_h], axis=0),
                np.concatenate([bq[hs], bk[hs]]).reshape(128, 1).astype(np.float32),
                np.concatenate(
                    [bv[hs].astype(np.float32), np.zeros(HD, np.float32)]
                ).reshape(128, 1),
            ],
            axis=1,
        ).astype(np.float32)
        in_maps.append(
            {
                "xt": xt,
                "wpack": np.ascontiguousarray(wpack),
                "bpack": bpack,
                "onesb": onesb,
            }
        )
    return in_maps


def _install_ntff_hook():
    """Register the axon NTFF profiling hook (test-only plumbing)."""
    import types

    try:
        from antenv.axon_hooks import set_axon_ntff_profile_hook  # noqa: F401
    except ImportError:
        m = types.ModuleType("antenv.axon_hooks")
        m._HOOK = None
        m.set_axon_ntff_profile_hook = lambda h: setattr(m, "_HOOK", h)
        m.get_axon_ntff_profile_hook = lambda: m._HOOK
        sys.modules["antenv.axon_hooks"] = m
        import antenv

        antenv.axon_hooks = m
    from antenv.axon_hooks import (
        get_axon_ntff_profile_hook,
        set_axon_ntff_profile_hook,
    )

    if get_axon_ntff_profile_hook() is None:
        import trn_agent_boot.trn_boot as tb

        set_axon_ntff_profile_hook(
            tb._ntff_profile_via_ctypes("/opt/axon/libaxon_pjrt.so")
        )


def kernel(x, Wq, bq, Wk, bk, Wv, bv, Wo, bo, _trace=False):
    x, Wq, bq, Wk, bk, Wv, bv, Wo, bo = (
        np.asarray(a, dtype=np.float32) for a in (x, Wq, bq, Wk, bk, Wv, bv, Wo, bo)
    )
    if "nc" not in _CACHE:
        _CACHE["nc"] = _build()
    nc = _CACHE["nc"]
    in_maps = _prep_inputs(x, Wq, bq, Wk, bk, Wv, bv, Wo, bo)
    kwargs = {}
    if _trace:
        _install_ntff_hook()
        kwargs = dict(trace=True, trace_cores=[0])
    res = run_bass_kernel_spmd(nc, in_maps, core_ids=list(range(8)), **kwargs)
    _CACHE["last_result"] = res
    y = np.zeros((TOK, D), dtype=np.float64)
    for r in res.results:
        y += r["y"].astype(np.float64) / r["l"].astype(np.float64)[:, None]
    y += bo[None, :]
    return y.astype(np.float32).reshape(B, S, D)


# revision 25
# speedup vs baseline: 1.4381x; 1.0176x over previous
"""Causal self-attention (B=2, S=4096, D=512, H=8) on 8 Trainium2 NeuronCores.

Sharding: tensor-parallel over heads. Core h computes head h for both batch
elements: QKV projections for its head, causal flash attention, and its
partial (unnormalized) o_proj contribution y_h = U_h @ Wo[h*64:(h+1)*64, :]
plus the per-query softmax denominators L_h. The host computes
sum_h(y_h / L_h) + bo.

Per-core structure (hd = 64, S = 4096, 32 k-tiles of 128 per batch):
  - xT [512, 8192] (host-pretransposed x) streams in as [128f, 4c, 512t]
    tiles; QK projection matmul (lhsT = [Wq_h | Wk_h] chunk) produces
    psum [Q.T; K.T] per 512-token block; V.T separately, then PE-transposed
    to V natural (bf16).
  - QT2 [128, 4096]/batch: Q.T duplicated in both partition halves (dup via
    SBUF->SBUF DMA). KT2 [128, 2048]/batch: K.T pair-packed -- even k-tiles
    in partitions 0:64, odd k-tiles in partitions 64:128.
  - Scores: per k-tile pair, TWO K=64 matmuls at tile_position (0,0) and
    (64,0) (disjoint PE row groups -> run concurrently): S.T chunk psum
    [128, 2, 512]. One ACT exp call [128, 1024] PSUM->SBUF produces P.T in
    bf16; diagonal chunks get a 0/1 causal mask multiply on DVE.
  - AV (bf16): U'[65, 512] += V'_kt.T @ P.T_kt with V' = [V | ones]; row 64
    accumulates L. AV for chunk j is emitted after the scores of chunk j+1.
  - o_proj: U dup'd to both partition halves; per q-subtile pair, TWO K=64
    matmuls at row groups 0/64 against Wo_h (host-duplicated into both
    halves); y out in bf16, 256 rows per DMA, unnormalized, plus L.
  - proj of block t+1 and o_proj of block t-1 are emitted as small "filler"
    pieces between attention chunks so the Scalar engine (exp, the
    throughput floor at ~1.15us per 1024-col chunk) never starves and the
    PE never idles long enough to re-throttle (HAM).

Matmuls: scores/projections/o_proj in float32r (~1.6e-4), AV in bf16.
"""

import sys

for _p in ("/opt/trn_rl_repo", "/root/.axon_site/_ro/trn_rl_repo"):
    if _p not in sys.path:
        sys.path.insert(0, _p)

from collections import deque

import numpy as np

import concourse.bass as bass
import concourse.mybir as mybir
import concourse.tile as tile
from concourse import bacc
from concourse.bass_utils import run_bass_kernel_spmd

B = 2
S = 4096
D = 512
H = 8
HD = 64
TOK = B * S          # 8192
NKT = S // 128       # 32 k-tiles per batch
SCALE = HD ** -0.5

F32 = mybir.dt.float32
F32R = mybir.dt.float32r
BF16 = mybir.dt.bfloat16

_CACHE = {}


def _build():
    nc = bacc.Bacc("TRN2", target_bir_lowering=False, debug=False, num_devices=8)

    xt_d = nc.dram_tensor("xt", [D, TOK], F32R, kind="ExternalInput")
    # all fp32 weights packed: wqk[512] | wv[256] | wo[512] | bqk[1] | bv[1]
    wpack_d = nc.dram_tensor("wpack", [128, 1282], F32R, kind="ExternalInput")
    # all bf16 constants packed: mask[2048] | identb[64] (rows 0:64)
    bpack_d = nc.dram_tensor("bpack", [128, 2112], BF16, kind="ExternalInput")
    onesb_d = nc.dram_tensor("onesb", [128, NKT], BF16, kind="ExternalInput")
    y_d = nc.dram_tensor("y", [TOK, D], BF16, kind="ExternalOutput")
    l_d = nc.dram_tensor("l", [TOK], F32R, kind="ExternalOutput")

    xt_r = xt_d.ap().rearrange("(c p) t -> p c t", p=128)      # [128, 4, 8192]

    with tile.TileContext(nc) as tc:
        import contextlib

        with contextlib.ExitStack() as ctx:
            singles = ctx.enter_context(tc.tile_pool(name="singles", bufs=1))
            xpool = ctx.enter_context(tc.tile_pool(name="xt", bufs=4))
            ptpool = ctx.enter_context(tc.tile_pool(name="pt", bufs=5))
            upool = ctx.enter_context(tc.tile_pool(name="usb", bufs=2))
            ypool = ctx.enter_context(tc.tile_pool(name="ysb", bufs=6))
            kstpool = ctx.enter_context(tc.tile_pool(name="kst", bufs=2))
            ktspool = ctx.enter_context(tc.tile_pool(name="kts", bufs=2))

            ps_st = ctx.enter_context(
                tc.tile_pool(name="ps_st", bufs=2, space="PSUM")
            )
            ps_u = ctx.enter_context(tc.tile_pool(name="ps_u", bufs=2, space="PSUM"))
            ps_misc = ctx.enter_context(
                tc.tile_pool(name="ps_misc", bufs=2, space="PSUM")
            )

            # --- persistent per-batch activation buffers ---------------
            qt2 = [
                singles.tile([128, S], F32R, tag=f"qt2_{b}", name=f"qt2_{b}")
                for b in range(B)
            ]
            kt2 = [
                singles.tile([128, S // 2], F32R, tag=f"kt2_{b}", name=f"kt2_{b}")
                for b in range(B)
            ]
            vp = [
                singles.tile([128, NKT * 65], BF16, tag=f"vp_{b}", name=f"vp_{b}")
                for b in range(B)
            ]
            l_acc = singles.tile([1, TOK], F32R, name="l_acc")

            # batch-interleaved order: all ACT-starved small blocks land at
            # the start (where exp work is scarce anyway); no mid-stream ramp
            blocks = [(b, t) for t in range(8) for b in range(B)]
            xt_tiles = {}

            def start_xt(i):
                if i >= len(blocks):
                    return
                b, tb = blocks[i]
                t0 = b * S + tb * 512
                xt_sb = xpool.tile([128, 4, 512], F32R, tag="xt")
                nc.gpsimd.dma_start(out=xt_sb, in_=xt_r[:, :, t0 : t0 + 512])
                xt_tiles[i] = xt_sb

            # --- constants / weights (xt(0) first: it gates proj(0)) ---
            start_xt(0)
            wpack_sb = singles.tile([128, 1282], F32R, name="wpack_sb")
            bpack_sb = singles.tile([128, 2112], BF16, name="bpack_sb")
            ones_sb = singles.tile([128, NKT], BF16, name="ones_sb")
            nc.sync.dma_start(out=wpack_sb, in_=wpack_d.ap())
            nc.sync.dma_start(out=bpack_sb, in_=bpack_d.ap())
            nc.sync.dma_start(out=ones_sb, in_=onesb_d.ap())
            start_xt(1)

            def wqk_sb(c):  # [128, 128] lhsT chunk c of [Wq_h | Wk_h]
                return wpack_sb[:, c * 128 : (c + 1) * 128]

            def wv_sbc(c):  # [128, 64] lhsT chunk c of Wv_h
                return wpack_sb[:, 512 + c * 64 : 512 + (c + 1) * 64]

            wo_sb = wpack_sb[:, 768:1280]      # [128, 512] Wo_h dup'd rows
            bqk_sb = wpack_sb[:, 1280:1281].bitcast(F32)    # [128, 1]
            bv_sb = wpack_sb[0:64, 1281:1282].bitcast(F32)  # [64, 1]
            identb = bpack_sb[0:64, 2048:2112]

            def vp_ones(b):
                nc.vector.tensor_copy(
                    vp[b].rearrange("p (t c) -> p t c", c=65)[:, :, 64:65],
                    ones_sb.rearrange("p (t c) -> p t c", c=1),
                )

            def proj_pieces(i):
                """Projections for block i as a list of small emit-closures."""
                b, tb = blocks[i]
                st_ = {}

                def qk(c0):
                    def f():
                        if c0 == 0:
                            st_["qk"] = ps_misc.tile(
                                [128, 512], F32, tag="m", name="qkps"
                            )
                        for c in (c0, c0 + 1):
                            nc.tensor.matmul(
                                st_["qk"],
                                wqk_sb(c),
                                xt_tiles[i][:, c, :],
                                start=(c == 0),
                                stop=(c == 3),
                                skip_group_check=True,
                            )
                    return f

                def qk_ex():
                    qk_ps = st_["qk"]
                    cols = slice(tb * 512, (tb + 1) * 512)
                    nc.vector.tensor_scalar_add(
                        qt2[b][0:64, cols], qk_ps[0:64, :], bqk_sb[0:64, 0:1]
                    )
                    nc.sync.dma_start(
                        out=qt2[b][64:128, cols], in_=qt2[b][0:64, cols]
                    )
                    kts = ktspool.tile([128, 512], F32R, tag="kts")
                    nc.vector.tensor_scalar_add(
                        kts[64:128, :], qk_ps[64:128, :], bqk_sb[64:128, 0:1]
                    )
                    # pair-pack K.T: even k-tile -> rows 0:64, odd -> 64:128
                    kts_r = kts[64:128, :].rearrange(
                        "p (j1 j2 c) -> p j1 j2 c", j2=2, c=128
                    )
                    c0 = 256 * tb
                    nc.sync.dma_start(
                        out=kt2[b][0:64, c0 : c0 + 256].rearrange(
                            "p (j1 one c) -> p j1 one c", one=1, c=128
                        ),
                        in_=kts_r[:, :, 0:1, :],
                    )
                    nc.sync.dma_start(
                        out=kt2[b][64:128, c0 : c0 + 256].rearrange(
                            "p (j1 one c) -> p j1 one c", one=1, c=128
                        ),
                        in_=kts_r[:, :, 1:2, :],
                    )

                def vv(c0):
                    def f():
                        if c0 == 0:
                            st_["v"] = ps_misc.tile(
                                [128, 512], F32, tag="m", name="vps"
                            )
                        for c in (c0, c0 + 1):
                            nc.tensor.matmul(
                                st_["v"][0:HD, :],
                                wv_sbc(c),
                                xt_tiles[i][:, c, :],
                                start=(c == 0),
                                stop=(c == 3),
                                skip_group_check=True,
                            )
                    return f

                def v_ex():
                    vt_sb = kstpool.tile([HD, 512], BF16, tag="vt")
                    st_["vt"] = vt_sb
                    nc.vector.tensor_scalar_add(
                        vt_sb, st_["v"][0:HD, :], bv_sb[:, 0:1]
                    )

                def tr(jj):
                    def f():
                        for j in (jj, jj + 1):
                            kt = tb * 4 + j
                            vtr_ps = ps_misc.tile([128, HD], BF16, tag="m")
                            nc.tensor.transpose(
                                vtr_ps, st_["vt"][:, j * 128 : (j + 1) * 128], identb
                            )
                            nc.vector.tensor_copy(
                                vp[b][:, kt * 65 : kt * 65 + 64], vtr_ps
                            )
                    return f

                return [qk(0), qk(2), qk_ex, vv(0), vv(2), v_ex, tr(0), tr(2)]

            def attn_qblock(b, qb, fq):
                """Attention for q-block qb; pops one filler per chunk."""
                q0 = qb * 512
                if fq:  # prev block's u_ps consumers must emit before realloc
                    fq.popleft()[1]()
                u_ps = ps_u.tile([65, 512], F32, tag="u")
                n_chunks = 2 * (qb + 1)
                n_kt = 2 * n_chunks

                def emit_av(pt, j):
                    for j2 in range(2):
                        kt = 2 * j + j2
                        nc.tensor.matmul(
                            u_ps,
                            vp[b][:, kt * 65 : kt * 65 + 65],
                            pt[:, j2, :],
                            start=(kt == 0),
                            stop=(kt == n_kt - 1),
                            skip_group_check=True,
                        )

                prev_pt = None
                for j in range(n_chunks):
                    st = ps_st.tile([128, 2, 512], F32, tag="st")
                    nc.tensor.matmul(
                        st[:, 0, :],
                        kt2[b][0:64, j * 128 : (j + 1) * 128],
                        qt2[b][0:64, q0 : q0 + 512],
                        start=True,
                        stop=True,
                        tile_position=(0, 0),
                    )
                    nc.tensor.matmul(
                        st[:, 1, :],
                        kt2[b][64:128, j * 128 : (j + 1) * 128],
                        qt2[b][64:128, q0 : q0 + 512],
                        start=True,
                        stop=True,
                        tile_position=(64, 0),
                    )
                    pt = ptpool.tile([128, 2, 512], BF16, tag="pt")
                    nc.scalar.activation(
                        pt, st, mybir.ActivationFunctionType.Exp, scale=SCALE
                    )
                    if j >= n_chunks - 2:  # diagonal chunks: causal mask
                        d0 = (j % 2) * 2
                        ptf = pt.rearrange("p a b -> p (a b)")
                        nc.vector.tensor_mul(
                            ptf, ptf, bpack_sb[:, d0 * 512 : (d0 + 2) * 512]
                        )
                    if prev_pt is not None:
                        emit_av(prev_pt, j - 1)
                    prev_pt = pt
                    if fq:
                        fq.popleft()[1]()
                emit_av(prev_pt, n_chunks - 1)
                return u_ps

            def fin_pieces(b, qb, u_ps):
                """U->SBUF, L out, row-packed o_proj pairs, y out (bf16)."""
                row0 = b * S + qb * 512
                st_ = {}

                def p1():
                    u_sb = upool.tile([128, 512], F32R, tag="u")
                    st_["u"] = u_sb
                    nc.vector.tensor_copy(u_sb[0:64, :], u_ps[0:64, :])
                    nc.vector.tensor_copy(
                        l_acc[0:1, row0 : row0 + 512], u_ps[64:65, :]
                    )
                    nc.sync.dma_start(out=u_sb[64:128, :], in_=u_sb[0:64, :])

                def half(jp, k):
                    def f():
                        u_sb = st_["u"]
                        if k == 0:
                            st_[f"y{jp}"] = ypool.tile(
                                [128, 2, 512], BF16, tag="y", name=f"ysb{jp}"
                            )
                        ysb = st_[f"y{jp}"]
                        j2 = 2 * jp + k
                        y_ps = ps_misc.tile([128, 512], F32, tag="m", name="yps")
                        nc.tensor.matmul(
                            y_ps,
                            u_sb[k * 64 : (k + 1) * 64, j2 * 128 : (j2 + 1) * 128],
                            wo_sb[k * 64 : (k + 1) * 64, :],
                            start=True,
                            stop=True,
                            tile_position=(k * 64, 0),
                        )
                        nc.vector.tensor_copy(ysb[:, k, :], y_ps)
                        if k == 1:
                            r0 = row0 + jp * 256
                            nc.gpsimd.dma_start(
                                out=y_d.ap()[r0 : r0 + 256, :].rearrange(
                                    "(j p) d -> p j d", p=128
                                ),
                                in_=ysb,
                            )
                    return f

                return [p1, half(0, 0), half(0, 1), half(1, 0), half(1, 1)]

            # --- main pipeline -----------------------------------------
            # filler items are (deadline_iter, fn): a piece must be emitted
            # by the END of its deadline iteration; until then it spreads
            # into chunk slack (one pop per attention chunk).
            fq = deque()
            for piece in proj_pieces(0):
                piece()
            vp_ones(0)
            start_xt(2)
            fq.append((0, lambda: vp_ones(1)))
            fq.extend((0, p) for p in proj_pieces(1))
            for i, (b, t) in enumerate(blocks):
                start_xt(i + 3)
                if i + 2 < len(blocks):
                    fq.extend((i + 1, p) for p in proj_pieces(i + 2))
                u_ps = attn_qblock(b, t, fq)
                while fq and fq[0][0] <= i:
                    fq.popleft()[1]()
                fq.extend((i + 1, p) for p in fin_pieces(b, t, u_ps))
            while fq:
                fq.popleft()[1]()
            nc.sync.dma_start(
                out=l_d.ap().rearrange("(p c) -> p c", p=1), in_=l_acc
            )

    nc.compile()
    return nc


def _prep_inputs(x, Wq, bq, Wk, bk, Wv, bv, Wo, bo):
    import ml_dtypes

    xt = np.ascontiguousarray(x.reshape(TOK, D).T).astype(np.float32)
    mask = np.zeros((128, 4, 512), dtype=np.float32)
    p = np.arange(128)[:, None]
    c = np.arange(512)[None, :]
    for d in range(4):
        mask[:, d, :] = (p + 128 * d <= c).astype(np.float32)
    mask = mask.astype(ml_dtypes.bfloat16)
    identb = np.eye(64, dtype=np.float32).astype(ml_dtypes.bfloat16)
    onesb = np.ones((128, NKT), dtype=np.float32).astype(ml_dtypes.bfloat16)

    bpack = np.concatenate(
        [
            mask.astype(np.float32).reshape(128, 2048),
            np.concatenate([identb.astype(np.float32), np.zeros((64, 64), np.float32)], axis=0),
        ],
        axis=1,
    ).astype(ml_dtypes.bfloat16)

    in_maps = []
    for h in range(H):
        hs = slice(h * HD, (h + 1) * HD)
        wo_h = np.ascontiguousarray(Wo[hs, :]).astype(np.float32)
        wqk = np.concatenate([Wq[:, hs], Wk[:, hs]], axis=1).astype(np.float32)
        wv_h = np.asarray(Wv[:, hs], dtype=np.float32)
        wpack = np.concatenate(
            [
                wqk.reshape(4, 128, 128).transpose(1, 0, 2).reshape(128, 512),
                wv_h.reshape(4, 128, HD).transpose(1, 0, 2).reshape(128, 256),
                np.concatenate([wo_h, wo_h], axis=0),
                np.concatenate([bq[hs], bk[hs]]).reshape(128, 1).astype(np.float32),
                np.concatenate(
                    [bv[hs].astype(np.float32), np.zeros(HD, np.float32)]
                ).reshape(128, 1),
            ],
            axis=1,
        ).astype(np.float32)
        in_maps.append(
            {
                "xt": xt,
                "wpack": np.ascontiguousarray(wpack),
                "bpack": bpack,
                "onesb": onesb,
            }
        )
    return in_maps


def _install_ntff_hook():
    """Register the axon NTFF profiling hook (test-only plumbing)."""
    import types

    try:
        from antenv.axon_hooks import set_axon_ntff_profile_hook  # noqa: F401
    except ImportError:
        m = types.ModuleType("antenv.axon_hooks")
        m._HOOK = None
        m.set_axon_ntff_profile_hook = lambda h: setattr(m, "_HOOK", h)
        m.get_axon_ntff_profile_hook = lambda: m._HOOK
        sys.modules["antenv.axon_hooks"] = m
        import antenv

        antenv.axon_hooks = m
    from antenv.axon_hooks import (
        get_axon_ntff_profile_hook,
        set_axon_ntff_profile_hook,
    )

    if get_axon_ntff_profile_hook() is None:
        import trn_agent_boot.trn_boot as tb

        set_axon_ntff_profile_hook(
            tb._ntff_profile_via_ctypes("/opt/axon/libaxon_pjrt.so")
        )


def kernel(x, Wq, bq, Wk, bk, Wv, bv, Wo, bo, _trace=False):
    x, Wq, bq, Wk, bk, Wv, bv, Wo, bo = (
        np.asarray(a, dtype=np.float32) for a in (x, Wq, bq, Wk, bk, Wv, bv, Wo, bo)
    )
    if "nc" not in _CACHE:
        _CACHE["nc"] = _build()
    nc = _CACHE["nc"]
    in_maps = _prep_inputs(x, Wq, bq, Wk, bk, Wv, bv, Wo, bo)
    kwargs = {}
    if _trace:
        _install_ntff_hook()
        kwargs = dict(trace=True, trace_cores=[0])
    res = run_bass_kernel_spmd(nc, in_maps, core_ids=list(range(8)), **kwargs)
    _CACHE["last_result"] = res
    y = np.zeros((TOK, D), dtype=np.float64)
    for r in res.results:
        y += r["y"].astype(np.float64) / r["l"].astype(np.float64)[:, None]
    y += bo[None, :]
    return y.astype(np.float32).reshape(B, S, D)
